# revision 14
# baseline (speedup 1.0000x reference)
"""Causal attention kernel for Trainium2 (Bass/Tile), batch-sharded over 8 cores.

Reference computation (per batch b):
    S = Q @ K^T                  [S, S]
    S -= triu(ones, k=1) * 1e10  (causal mask, applied before scaling)
    P = softmax(S / sqrt(512), axis=-1)
    O = P @ V                    [S, D]

Shapes: B=16, S=2048, D=512, fp32. Each of the 8 cores handles 2 batches.

Design notes:
  - All matmuls run as float32r (full-rate PE; fp32 would be 4x slower).
  - S^T layout ([keys, queries]) so the exp output P^T feeds the PV matmul
    directly as the stationary operand; no per-tile transposes of P.
  - No max-subtraction in the softmax: logits after scaling are ~N(0,1)
    (|logit| < ~7), exp cannot overflow in fp32.
  - Row sums of P come from an extra N=1 matmul per (i,j) pair reusing the
    already-loaded P^T stationary block against a ones vector.
  - Q^T / K^T are built on-chip with PE transposes (d must sit on partitions
    for both QK^T operands).
"""

import sys

sys.path.insert(0, "/opt/trn_rl_repo")

from contextlib import ExitStack

import numpy as np

import concourse.bass as bass
import concourse.tile as tile
from concourse import bacc, mybir
from concourse.bass_utils import run_bass_kernel_spmd
from concourse.masks import make_identity

N_CORES = 8
B_FULL = 16
B_LOC = B_FULL // N_CORES  # batches per core
S = 2048
D = 512
P = 128  # partitions
DC = D // P  # d-chunks (4)
NKB = S // P  # key blocks per batch (16)
NG = S // 512  # query groups of 512 (4)
SCALE = 1.0 / np.sqrt(np.float32(D))  # 1/22.627
MASK_VAL = -1e9

F32 = mybir.dt.float32
F32R = mybir.dt.float32r


def _build_attention(ctx: ExitStack, tc: tile.TileContext, out_ap, q_ap, k_ap, v_ap):
    nc = tc.nc

    consts = ctx.enter_context(tc.tile_pool(name="consts", bufs=1))
    stage = ctx.enter_context(tc.tile_pool(name="stage", bufs=6))
    kt_pool = ctx.enter_context(tc.tile_pool(name="kt", bufs=1))
    qt_pool = ctx.enter_context(tc.tile_pool(name="qt", bufs=2))
    v_pool = ctx.enter_context(tc.tile_pool(name="v", bufs=1))
    pt_pool = ctx.enter_context(tc.tile_pool(name="pt", bufs=2))
    o_pool = ctx.enter_context(tc.tile_pool(name="o", bufs=3))
    small = ctx.enter_context(tc.tile_pool(name="small", bufs=4))
    ps_st = ctx.enter_context(tc.tile_pool(name="ps_st", bufs=2, space="PSUM"))
    ps_tp = ctx.enter_context(tc.tile_pool(name="ps_tp", bufs=2, space="PSUM"))
    ps_o1 = ctx.enter_context(tc.tile_pool(name="ps_o1", bufs=2, space="PSUM"))
    ps_o2 = ctx.enter_context(tc.tile_pool(name="ps_o2", bufs=2, space="PSUM"))

    # Identity (for PE transpose) and causal mask for diagonal blocks.
    ident = consts.tile([P, P], F32)
    make_identity(nc, ident)
    # S^T orientation: entry [kk, qq] is masked (add -1e9) when kk > qq,
    # i.e. strictly below the diagonal (partition index > free index).
    mask = consts.tile([P, P], F32)
    nc.gpsimd.memset(mask, 0.0)
    nc.gpsimd.affine_select(
        out=mask,
        in_=mask,
        compare_op=mybir.AluOpType.is_ge,
        fill=MASK_VAL,
        base=0,
        # keep 0.0 where (-kk + qq) >= 0, else fill MASK_VAL
        pattern=[[1, P]],
        channel_multiplier=-1,
    )
    # Softmax denominators come from two ones-columns prepended to V: the PV
    # matmul then yields [sum, sum, O...] in one accumulation (two columns
    # because fp32r matmuls need an even moving free dim).
    ones_f = consts.tile([P, 2], F32)
    nc.vector.memset(ones_f, 1.0)

    for b in range(B_LOC):
        # ---- Stage 0: load V (ones-augmented); prefetch K/Q staging ---------
        # v_sb[:, j, 0:2] = 1.0 (softmax-denominator columns), [:, j, 2:] = V.
        v_sb = v_pool.tile([P, NKB, D + 2], F32R)
        for kb in range(NKB):
            nc.vector.tensor_copy(v_sb[:, kb, 0:2], ones_f)
        vchunk = max(1, NKB // 4)  # chunks so early key blocks arrive fast
        for vc in range(0, NKB, vchunk):
            nc.gpsimd.dma_start(
                out=v_sb[:, vc : vc + vchunk, 2:],
                in_=v_ap[b, vc * P : (vc + vchunk) * P, :].rearrange(
                    "(kb p) d -> p kb d", p=P
                ),
            )

        # Prefetch all natural-layout K and Q row blocks up front.
        knats = []
        qnats = []
        for kb in range(NKB):
            knat = stage.tile([P, D], F32, tag="knat")
            nc.sync.dma_start(out=knat, in_=k_ap[b, kb * P : (kb + 1) * P, :])
            knats.append(knat)
        for qb in range(NKB):
            qnat = stage.tile([P, D], F32, tag="qnat")
            nc.sync.dma_start(out=qnat, in_=q_ap[b, qb * P : (qb + 1) * P, :])
            qnats.append(qnat)

        # ---- Build K^T via PE transposes ------------------------------------
        kt = kt_pool.tile([P, DC, S], F32R)  # [d_part, dc, keys]
        for kb in range(NKB):
            tp = ps_tp.tile([P, DC, P], F32)
            for dc in range(DC):
                nc.tensor.transpose(
                    tp[:, dc, :], knats[kb][:, dc * P : (dc + 1) * P], ident
                )
            nc.vector.tensor_copy(kt[:, :, kb * P : (kb + 1) * P], tp)

        for g in range(NG):
            # ---- Build Q^T for this query group (512 queries) ---------------
            qt = qt_pool.tile([P, DC, 512], F32R)  # [d_part, dc, q_local]
            for t in range(4):
                qb = 4 * g + t
                tp = ps_tp.tile([P, DC, P], F32)
                for dc in range(DC):
                    nc.tensor.transpose(
                        tp[:, dc, :], qnats[qb][:, dc * P : (dc + 1) * P], ident
                    )
                nc.vector.tensor_copy(qt[:, :, t * P : (t + 1) * P], tp)

            # ---- Phase A: S^T = K^T.T @ Q^T per key block; mask; exp --------
            pt = pt_pool.tile([P, NKB, 512], F32R)  # [k_part, j, q_local]
            for j in range(4 * g + 4):
                o_off = max(0, (j - 4 * g) * P)  # first allowed local query
                w = 512 - o_off
                st = ps_st.tile([P, 512], F32)
                for dc in range(DC):
                    nc.tensor.matmul(
                        st[:, :w],
                        kt[:, dc, j * P : (j + 1) * P],
                        qt[:, dc, o_off:512],
                        start=(dc == 0),
                        stop=(dc == DC - 1),
                    )
                if j >= 4 * g:  # diagonal block: in-block causal mask
                    nc.vector.tensor_add(st[:, 0:P], st[:, 0:P], mask)
                nc.scalar.activation(
                    pt[:, j, o_off:512],
                    st[:, :w],
                    mybir.ActivationFunctionType.Exp,
                    bias=0.0,
                    scale=float(SCALE),
                )

            # ---- Phase B: [sums|O] = P^T.T @ [1|V]; normalize; store ---------
            # Split into N=258 and N=256 matmuls (one PSUM bank each); the
            # first two columns of bank 1 are the softmax denominators.
            for t in range(4):
                i = 4 * g + t  # global query tile
                o1 = ps_o1.tile([P, 258], F32)
                o2 = ps_o2.tile([P, 256], F32)
                for j in range(i + 1):
                    lhsT = pt[:, j, t * P : (t + 1) * P]
                    nc.tensor.matmul(
                        o1, lhsT, v_sb[:, j, 0:258], start=(j == 0), stop=(j == i)
                    )
                    nc.tensor.matmul(
                        o2, lhsT, v_sb[:, j, 258:514], start=(j == 0), stop=(j == i)
                    )
                recip = small.tile([P, 1], F32)
                nc.vector.reciprocal(recip, o1[:, 0:1])
                o_sb = o_pool.tile([P, D], F32)
                nc.vector.tensor_scalar_mul(o_sb[:, 0:256], o1[:, 2:258], recip)
                nc.vector.tensor_scalar_mul(o_sb[:, 256:512], o2, recip)
                nc.sync.dma_start(
                    out=out_ap[b, i * P : (i + 1) * P, :], in_=o_sb
                )


def build_nc():
    nc = bacc.Bacc(None, target_bir_lowering=False, debug=False)
    q = nc.dram_tensor("query", [B_LOC, S, D], F32, kind="ExternalInput").ap()
    k = nc.dram_tensor("key", [B_LOC, S, D], F32, kind="ExternalInput").ap()
    v = nc.dram_tensor("value", [B_LOC, S, D], F32, kind="ExternalInput").ap()
    out = nc.dram_tensor("out", [B_LOC, S, D], F32, kind="ExternalOutput").ap()
    with tile.TileContext(nc) as tc:
        with ExitStack() as ctx:
            _build_attention(ctx, tc, out, q, k, v)
    nc.compile()
    return nc


def kernel(query, key, value, _trace=False):
    query = np.ascontiguousarray(query, dtype=np.float32)
    key = np.ascontiguousarray(key, dtype=np.float32)
    value = np.ascontiguousarray(value, dtype=np.float32)
    nc = build_nc()
    in_maps = [
        {
            "query": query[c * B_LOC : (c + 1) * B_LOC],
            "key": key[c * B_LOC : (c + 1) * B_LOC],
            "value": value[c * B_LOC : (c + 1) * B_LOC],
        }
        for c in range(N_CORES)
    ]
    res = run_bass_kernel_spmd(nc, in_maps, list(range(N_CORES)), trace=_trace)
    out = np.concatenate([res.results[c]["out"] for c in range(N_CORES)], axis=0)
    if _trace:
        return out, res
    return out


# revision 17
# speedup vs baseline: 1.3346x; 1.3346x over previous
"""Causal attention kernel for Trainium2 (Bass/Tile), batch-sharded over 8 cores.

Reference computation (per batch b):
    S = Q @ K^T                  [S, S]
    S -= triu(ones, k=1) * 1e10  (causal mask, applied before scaling)
    P = softmax(S / sqrt(512), axis=-1)
    O = P @ V                    [S, D]

Shapes: B=16, S=2048, D=512, fp32. Each of the 8 cores handles 2 batches.

Design notes:
  - Matmul operands are fp16 (accumulation in PSUM stays fp32): fp32/fp32r
    matmuls pay a serialized internal weight load per instruction (~107ns),
    which dominated with many 128-wide stationary blocks. fp16 matmuls run at
    the same PE rate with a separately-pipelined weight load. fp16 over bf16
    for its 10-bit mantissa; all intermediate ranges (logits <~8 after scale,
    exp <~250, inputs ~N(0,1)) fit fp16 comfortably.
  - S^T layout ([keys, queries]) so the exp output P^T feeds the PV matmul
    directly as the stationary operand; no per-tile transposes of P.
  - No max-subtraction in the softmax: logits after scaling are ~N(0,1)
    (|logit| < ~7), exp cannot overflow in fp32.
  - Softmax denominators come from two ones-columns prepended to V; the PV
    accumulation produces [sum, sum, O[:, :256]] + [O[:, 256:]] in two PSUM
    banks (one matmul each, fp32 bank limit is 512 columns).
  - The in-block causal mask is applied by an extra accumulating matmul
    (mask.T @ I) instead of a DVE pass over PSUM.
  - Q^T / K^T are built on-chip with PE transposes (d must sit on partitions
    for both QK^T operands).
"""

import sys

sys.path.insert(0, "/opt/trn_rl_repo")

from contextlib import ExitStack

import numpy as np

import concourse.bass as bass
import concourse.tile as tile
from concourse import bacc, mybir
from concourse.bass_utils import run_bass_kernel_spmd
from concourse.masks import make_causal_mask, make_identity

N_CORES = 8
B_FULL = 16
B_LOC = B_FULL // N_CORES  # batches per core
S = 2048
D = 512
P = 128  # partitions
DC = D // P  # d-chunks (4)
NKB = S // P  # key blocks per batch (16)
NG = S // 512  # query groups of 512 (4)
SCALE = 1.0 / np.sqrt(np.float32(D))  # 1/22.627
MASK_VAL = -60000.0  # fits fp16; -60000/22.6 -> exp underflows to 0

F32 = mybir.dt.float32
BF16 = mybir.dt.float16  # fp16: same PE rate as bf16, 4x finer mantissa


def _build_attention(ctx: ExitStack, tc: tile.TileContext, out_ap, q_ap, k_ap, v_ap):
    nc = tc.nc

    consts = ctx.enter_context(tc.tile_pool(name="consts", bufs=1))
    stage = ctx.enter_context(tc.tile_pool(name="stage", bufs=6))
    kt_pool = ctx.enter_context(tc.tile_pool(name="kt", bufs=1))
    qt_pool = ctx.enter_context(tc.tile_pool(name="qt", bufs=2))
    v_pool = ctx.enter_context(tc.tile_pool(name="v", bufs=2))
    pt_pool = ctx.enter_context(tc.tile_pool(name="pt", bufs=2))
    o_pool = ctx.enter_context(tc.tile_pool(name="o", bufs=4))
    small = ctx.enter_context(tc.tile_pool(name="small", bufs=4))
    ps_st = ctx.enter_context(tc.tile_pool(name="ps_st", bufs=2, space="PSUM"))
    ps_tp = ctx.enter_context(tc.tile_pool(name="ps_tp", bufs=2, space="PSUM"))
    ps_o1 = ctx.enter_context(tc.tile_pool(name="ps_o1", bufs=2, space="PSUM"))
    ps_o2 = ctx.enter_context(tc.tile_pool(name="ps_o2", bufs=2, space="PSUM"))

    # Identity for PE transposes (bf16, matching the staged data) and the
    # in-block causal mask, applied as an accumulating matmul U.T @ I, which
    # adds U[qq, kk] to S^T[kk, qq]; U is strictly-upper-triangular MASK_VAL
    # (mask where key kk > query qq).
    ident = consts.tile([P, P], BF16)
    make_identity(nc, ident)
    umask = consts.tile([P, P], BF16)
    make_causal_mask(nc, umask, mask_val=MASK_VAL)

    for b in range(B_LOC):
        # ---- Stage 0: load V (ones-augmented); prefetch K/Q staging ---------
        # v_sb[:, j, 0:2] = 1.0 (softmax-denominator columns), [:, j, 2:] = V.
        v_sb = v_pool.tile([P, NKB, D + 2], BF16)
        for kb in range(NKB):
            nc.vector.memset(v_sb[:, kb, 0:2], 1.0)
        vchunk = max(1, NKB // 4)  # chunks so early key blocks arrive fast
        for vc in range(0, NKB, vchunk):
            nc.gpsimd.dma_start(
                out=v_sb[:, vc : vc + vchunk, 2:],
                in_=v_ap[b, vc * P : (vc + vchunk) * P, :].rearrange(
                    "(kb p) d -> p kb d", p=P
                ),
            )

        # Prefetch all natural-layout K and Q row blocks (cast to bf16).
        knats = []
        qnats = []
        for kb in range(NKB):
            knat = stage.tile([P, D], BF16, tag="knat")
            nc.gpsimd.dma_start(out=knat, in_=k_ap[b, kb * P : (kb + 1) * P, :])
            knats.append(knat)
        for qb in range(NKB):
            qnat = stage.tile([P, D], BF16, tag="qnat")
            nc.gpsimd.dma_start(out=qnat, in_=q_ap[b, qb * P : (qb + 1) * P, :])
            qnats.append(qnat)

        # ---- Build K^T via PE transposes ------------------------------------
        kt = kt_pool.tile([P, DC, S], BF16)  # [d_part, dc, keys]
        for kb in range(NKB):
            tp = ps_tp.tile([P, DC, P], BF16)
            for dc in range(DC):
                nc.tensor.transpose(
                    tp[:, dc, :], knats[kb][:, dc * P : (dc + 1) * P], ident
                )
            nc.vector.tensor_copy(kt[:, :, kb * P : (kb + 1) * P], tp)

        for g in range(NG):
            # ---- Build Q^T for this query group (512 queries) ---------------
            qt = qt_pool.tile([P, DC, 512], BF16)  # [d_part, dc, q_local]
            for t in range(4):
                qb = 4 * g + t
                tp = ps_tp.tile([P, DC, P], BF16)
                for dc in range(DC):
                    nc.tensor.transpose(
                        tp[:, dc, :], qnats[qb][:, dc * P : (dc + 1) * P], ident
                    )
                nc.vector.tensor_copy(qt[:, :, t * P : (t + 1) * P], tp)

            # ---- Phase A: S^T = K^T.T @ Q^T per key block; mask; exp --------
            pt = pt_pool.tile([P, NKB, 512], BF16)  # [k_part, j, q_local]
            for j in range(4 * g + 4):
                o_off = max(0, (j - 4 * g) * P)  # first allowed local query
                w = 512 - o_off
                st = ps_st.tile([P, 512], F32)
                diag = j >= 4 * g
                for dc in range(DC):
                    nc.tensor.matmul(
                        st[:, :w],
                        kt[:, dc, j * P : (j + 1) * P],
                        qt[:, dc, o_off:512],
                        start=(dc == 0),
                        stop=(dc == DC - 1 and not diag),
                    )
                if diag:  # in-block causal mask via accumulating matmul
                    nc.tensor.matmul(
                        st[:, 0:P], umask, ident, start=False, stop=True
                    )
                nc.scalar.activation(
                    pt[:, j, o_off:512],
                    st[:, :w],
                    mybir.ActivationFunctionType.Exp,
                    bias=0.0,
                    scale=float(SCALE),
                )

            # ---- Phase B: [sums|O] = P^T.T @ [1|V]; normalize; store ---------
            for t in range(4):
                i = 4 * g + t  # global query tile
                o1 = ps_o1.tile([P, 258], F32)
                o2 = ps_o2.tile([P, 256], F32)
                for j in range(i + 1):
                    lhsT = pt[:, j, t * P : (t + 1) * P]
                    nc.tensor.matmul(
                        o1, lhsT, v_sb[:, j, 0:258], start=(j == 0), stop=(j == i)
                    )
                    nc.tensor.matmul(
                        o2, lhsT, v_sb[:, j, 258:514], start=(j == 0), stop=(j == i)
                    )
                recip = small.tile([P, 1], F32)
                nc.vector.reciprocal(recip, o1[:, 0:1])
                o_sb = o_pool.tile([P, D], F32)
                nc.vector.tensor_scalar_mul(o_sb[:, 0:256], o1[:, 2:258], recip)
                nc.vector.tensor_scalar_mul(o_sb[:, 256:512], o2, recip)
                nc.sync.dma_start(
                    out=out_ap[b, i * P : (i + 1) * P, :], in_=o_sb
                )


def build_nc():
    nc = bacc.Bacc(None, target_bir_lowering=False, debug=False)
    q = nc.dram_tensor("query", [B_LOC, S, D], F32, kind="ExternalInput").ap()
    k = nc.dram_tensor("key", [B_LOC, S, D], F32, kind="ExternalInput").ap()
    v = nc.dram_tensor("value", [B_LOC, S, D], F32, kind="ExternalInput").ap()
    out = nc.dram_tensor("out", [B_LOC, S, D], F32, kind="ExternalOutput").ap()
    with tile.TileContext(nc) as tc:
        with ExitStack() as ctx:
            _build_attention(ctx, tc, out, q, k, v)
    nc.compile()
    return nc


def kernel(query, key, value, _trace=False):
    query = np.ascontiguousarray(query, dtype=np.float32)
    key = np.ascontiguousarray(key, dtype=np.float32)
    value = np.ascontiguousarray(value, dtype=np.float32)
    nc = build_nc()
    in_maps = [
        {
            "query": query[c * B_LOC : (c + 1) * B_LOC],
            "key": key[c * B_LOC : (c + 1) * B_LOC],
            "value": value[c * B_LOC : (c + 1) * B_LOC],
        }
        for c in range(N_CORES)
    ]
    res = run_bass_kernel_spmd(nc, in_maps, list(range(N_CORES)), trace=_trace)
    out = np.concatenate([res.results[c]["out"] for c in range(N_CORES)], axis=0)
    if _trace:
        return out, res
    return out


# revision 20
# speedup vs baseline: 1.3984x; 1.0478x over previous
"""Causal attention kernel for Trainium2 (Bass/Tile), batch-sharded over 8 cores.

Reference computation (per batch b):
    S = Q @ K^T                  [S, S]
    S -= triu(ones, k=1) * 1e10  (causal mask, applied before scaling)
    P = softmax(S / sqrt(512), axis=-1)
    O = P @ V                    [S, D]

Shapes: B=16, S=2048, D=512, fp32. Each of the 8 cores handles 2 batches.

Design notes:
  - Matmul operands are fp16 (accumulation in PSUM stays fp32): fp32/fp32r
    matmuls pay a serialized internal weight load per instruction (~107ns),
    which dominated with many 128-wide stationary blocks. fp16 matmuls run at
    the same PE rate with a separately-pipelined weight load. fp16 over bf16
    for its 10-bit mantissa; all intermediate ranges (logits <~8 after scale,
    exp <~250, inputs ~N(0,1)) fit fp16 comfortably.
  - S^T layout ([keys, queries]) so the exp output P^T feeds the PV matmul
    directly as the stationary operand; no per-tile transposes of P.
  - No max-subtraction in the softmax: logits after scaling are ~N(0,1)
    (|logit| < ~7), exp cannot overflow in fp32.
  - Softmax denominators come from two ones-columns prepended to V; the PV
    accumulation produces [sum, sum, O[:, :256]] + [O[:, 256:]] in two PSUM
    banks (one matmul each, fp32 bank limit is 512 columns).
  - The in-block causal mask is applied by an extra accumulating matmul
    (mask.T @ I) instead of a DVE pass over PSUM.
  - Q^T / K^T are built on-chip with PE transposes (d must sit on partitions
    for both QK^T operands).
"""

import sys

sys.path.insert(0, "/opt/trn_rl_repo")

from contextlib import ExitStack

import numpy as np

import concourse.bass as bass
import concourse.tile as tile
from concourse import bacc, mybir
from concourse.bass_utils import run_bass_kernel_spmd
from concourse.masks import make_causal_mask, make_identity

N_CORES = 8
B_FULL = 16
B_LOC = B_FULL // N_CORES  # batches per core
S = 2048
D = 512
P = 128  # partitions
DC = D // P  # d-chunks (4)
NKB = S // P  # key blocks per batch (16)
NG = S // 512  # query groups of 512 (4)
SCALE = 1.0 / np.sqrt(np.float32(D))  # 1/22.627
MASK_VAL = -60000.0  # fits fp16; -60000/22.6 -> exp underflows to 0

F32 = mybir.dt.float32
BF16 = mybir.dt.float16  # fp16: same PE rate as bf16, 4x finer mantissa


def _build_attention(ctx: ExitStack, tc: tile.TileContext, out_ap, q_ap, k_ap, v_ap):
    nc = tc.nc

    consts = ctx.enter_context(tc.tile_pool(name="consts", bufs=1))
    stage = ctx.enter_context(tc.tile_pool(name="stage", bufs=6))
    kt_pool = ctx.enter_context(tc.tile_pool(name="kt", bufs=1))
    qt_pool = ctx.enter_context(tc.tile_pool(name="qt", bufs=2))
    v_pool = ctx.enter_context(tc.tile_pool(name="v", bufs=2))
    pt_pool = ctx.enter_context(tc.tile_pool(name="pt", bufs=2))
    o_pool = ctx.enter_context(tc.tile_pool(name="o", bufs=4))
    small = ctx.enter_context(tc.tile_pool(name="small", bufs=4))
    ps_st = ctx.enter_context(tc.tile_pool(name="ps_st", bufs=2, space="PSUM"))
    ps_tp = ctx.enter_context(tc.tile_pool(name="ps_tp", bufs=2, space="PSUM"))
    ps_o1 = ctx.enter_context(tc.tile_pool(name="ps_o1", bufs=2, space="PSUM"))
    ps_o2 = ctx.enter_context(tc.tile_pool(name="ps_o2", bufs=2, space="PSUM"))

    # Identity for PE transposes (bf16, matching the staged data) and the
    # in-block causal mask, applied as an accumulating matmul U.T @ I, which
    # adds U[qq, kk] to S^T[kk, qq]; U is strictly-upper-triangular MASK_VAL
    # (mask where key kk > query qq).
    ident = consts.tile([P, P], BF16)
    make_identity(nc, ident)
    umask = consts.tile([P, P], BF16)
    make_causal_mask(nc, umask, mask_val=MASK_VAL)

    for b in range(B_LOC):
        # ---- Stage 0: prefetch K, then Q[group 0], then V, then rest of Q ---
        # Ordering matters: everything shares the SWDGE ring, and the first PE
        # transposes need K blocks — don't queue 4MB of V ahead of them.
        # Staging tiles hold 4 row-blocks each ([p, kb, d] like V) so one
        # cast-DMA covers 4 blocks.
        knats = []  # chunk tiles of 4 key blocks each
        for kc in range(0, NKB, 4):
            knat = stage.tile([P, 4, D], BF16, tag="knat")
            nc.gpsimd.dma_start(
                out=knat,
                in_=k_ap[b, kc * P : (kc + 4) * P, :].rearrange(
                    "(kb p) d -> p kb d", p=P
                ),
            )
            knats.append(knat)

        qnats = []
        def _load_q_chunk(qc):
            qnat = stage.tile([P, 4, D], BF16, tag="qnat")
            nc.gpsimd.dma_start(
                out=qnat,
                in_=q_ap[b, qc * P : (qc + 4) * P, :].rearrange(
                    "(kb p) d -> p kb d", p=P
                ),
            )
            qnats.append(qnat)

        _load_q_chunk(0)  # group 0's queries, needed before V

        # v_sb[:, j, 0:2] = 1.0 (softmax-denominator columns), [:, j, 2:] = V.
        v_sb = v_pool.tile([P, NKB, D + 2], BF16)
        nc.vector.memset(v_sb[:, :, 0:2], 1.0)
        for vc in range(0, NKB, 4):
            nc.gpsimd.dma_start(
                out=v_sb[:, vc : vc + 4, 2:],
                in_=v_ap[b, vc * P : (vc + 4) * P, :].rearrange(
                    "(kb p) d -> p kb d", p=P
                ),
            )

        for qc in range(4, NKB, 4):
            _load_q_chunk(qc)

        # ---- Build K^T via PE transposes ------------------------------------
        kt = kt_pool.tile([P, DC, S], BF16)  # [d_part, dc, keys]
        for kb in range(NKB):
            tp = ps_tp.tile([P, DC, P], BF16)
            for dc in range(DC):
                nc.tensor.transpose(
                    tp[:, dc, :],
                    knats[kb // 4][:, kb % 4, dc * P : (dc + 1) * P],
                    ident,
                )
            nc.vector.tensor_copy(kt[:, :, kb * P : (kb + 1) * P], tp)

        for g in range(NG):
            # ---- Build Q^T for this query group (512 queries) ---------------
            qt = qt_pool.tile([P, DC, 512], BF16)  # [d_part, dc, q_local]
            for t in range(4):
                qb = 4 * g + t
                tp = ps_tp.tile([P, DC, P], BF16)
                for dc in range(DC):
                    nc.tensor.transpose(
                        tp[:, dc, :],
                        qnats[qb // 4][:, qb % 4, dc * P : (dc + 1) * P],
                        ident,
                    )
                nc.vector.tensor_copy(qt[:, :, t * P : (t + 1) * P], tp)

            # ---- Phase A: S^T = K^T.T @ Q^T per key block; mask; exp --------
            pt = pt_pool.tile([P, NKB, 512], BF16)  # [k_part, j, q_local]
            for j in range(4 * g + 4):
                o_off = max(0, (j - 4 * g) * P)  # first allowed local query
                w = 512 - o_off
                st = ps_st.tile([P, 512], F32)
                diag = j >= 4 * g
                for dc in range(DC):
                    nc.tensor.matmul(
                        st[:, :w],
                        kt[:, dc, j * P : (j + 1) * P],
                        qt[:, dc, o_off:512],
                        start=(dc == 0),
                        stop=(dc == DC - 1 and not diag),
                    )
                if diag:  # in-block causal mask via accumulating matmul
                    nc.tensor.matmul(
                        st[:, 0:P], umask, ident, start=False, stop=True
                    )
                nc.scalar.activation(
                    pt[:, j, o_off:512],
                    st[:, :w],
                    mybir.ActivationFunctionType.Exp,
                    bias=0.0,
                    scale=float(SCALE),
                )

            # ---- Phase B: [sums|O] = P^T.T @ [1|V]; normalize; store ---------
            for t in range(4):
                i = 4 * g + t  # global query tile
                o1 = ps_o1.tile([P, 258], F32)
                o2 = ps_o2.tile([P, 256], F32)
                for j in range(i + 1):
                    lhsT = pt[:, j, t * P : (t + 1) * P]
                    nc.tensor.matmul(
                        o1, lhsT, v_sb[:, j, 0:258], start=(j == 0), stop=(j == i)
                    )
                    nc.tensor.matmul(
                        o2, lhsT, v_sb[:, j, 258:514], start=(j == 0), stop=(j == i)
                    )
                recip = small.tile([P, 1], F32)
                nc.vector.reciprocal(recip, o1[:, 0:1])
                o_sb = o_pool.tile([P, D], F32)
                nc.vector.tensor_scalar_mul(o_sb[:, 0:256], o1[:, 2:258], recip)
                nc.vector.tensor_scalar_mul(o_sb[:, 256:512], o2, recip)
                nc.sync.dma_start(
                    out=out_ap[b, i * P : (i + 1) * P, :], in_=o_sb
                )


def build_nc():
    nc = bacc.Bacc(None, target_bir_lowering=False, debug=False)
    q = nc.dram_tensor("query", [B_LOC, S, D], F32, kind="ExternalInput").ap()
    k = nc.dram_tensor("key", [B_LOC, S, D], F32, kind="ExternalInput").ap()
    v = nc.dram_tensor("value", [B_LOC, S, D], F32, kind="ExternalInput").ap()
    out = nc.dram_tensor("out", [B_LOC, S, D], F32, kind="ExternalOutput").ap()
    with tile.TileContext(nc) as tc:
        with ExitStack() as ctx:
            _build_attention(ctx, tc, out, q, k, v)
    nc.compile()
    return nc


def kernel(query, key, value, _trace=False):
    query = np.ascontiguousarray(query, dtype=np.float32)
    key = np.ascontiguousarray(key, dtype=np.float32)
    value = np.ascontiguousarray(value, dtype=np.float32)
    nc = build_nc()
    in_maps = [
        {
            "query": query[c * B_LOC : (c + 1) * B_LOC],
            "key": key[c * B_LOC : (c + 1) * B_LOC],
            "value": value[c * B_LOC : (c + 1) * B_LOC],
        }
        for c in range(N_CORES)
    ]
    res = run_bass_kernel_spmd(nc, in_maps, list(range(N_CORES)), trace=_trace)
    out = np.concatenate([res.results[c]["out"] for c in range(N_CORES)], axis=0)
    if _trace:
        return out, res
    return out


# revision 21
# speedup vs baseline: 1.5267x; 1.0918x over previous
"""Causal attention kernel for Trainium2 (Bass/Tile), batch-sharded over 8 cores.

Reference computation (per batch b):
    S = Q @ K^T                  [S, S]
    S -= triu(ones, k=1) * 1e10  (causal mask, applied before scaling)
    P = softmax(S / sqrt(512), axis=-1)
    O = P @ V                    [S, D]

Shapes: B=16, S=2048, D=512, fp32. Each of the 8 cores handles 2 batches.

Design notes:
  - Matmul operands are fp16 (accumulation in PSUM stays fp32): fp32/fp32r
    matmuls pay a serialized internal weight load per instruction (~107ns),
    which dominated with many 128-wide stationary blocks. fp16 matmuls get a
    separately-pipelined LDWEIGHTS at the same 1 column/cycle stream rate.
    fp16 over bf16 for its 10-bit mantissa; all intermediate ranges (logits
    <~8 after scaling, exp <~250, inputs ~N(0,1)) fit fp16 comfortably.
  - S^T layout ([keys, queries]) so the exp output P^T feeds the PV matmul
    directly as the stationary operand; no per-tile transposes of P.
  - No max-subtraction in the softmax: logits after scaling are ~N(0,1)
    (|logit| < ~8), exp cannot overflow.
  - Softmax denominators come from two ones-columns prepended to V; the PV
    accumulation produces [sum, sum, O[:, :256]] + [O[:, 256:]] in two PSUM
    banks (fp32 bank limit is 512 columns per matmul).
  - The in-block causal mask is applied by an extra accumulating matmul
    (U.T @ I adds U[qq, kk] to S^T[kk, qq]) instead of a DVE pass over PSUM.
  - Q^T / K^T are built on-chip with PE transposes (d must sit on partitions
    for both QK^T operands); the next group's transposes are emitted before
    the current phase B so the PE fills the exp-wait bubble and the DVE
    copybacks stay ahead of the normalize ops.
  - All input DMAs are issued up front in need-order on the SWDGE ring
    (K first, then Q/V interleaved) — queueing V ahead of staging delayed
    the first PE work by ~20us in earlier revisions.
"""

import sys

sys.path.insert(0, "/opt/trn_rl_repo")

from contextlib import ExitStack

import numpy as np

import concourse.bass as bass
import concourse.tile as tile
from concourse import bacc, mybir
from concourse.bass_utils import run_bass_kernel_spmd
from concourse.masks import make_causal_mask, make_identity

N_CORES = 8
B_FULL = 16
B_LOC = B_FULL // N_CORES  # batches per core
S = 2048
D = 512
P = 128  # partitions
DC = D // P  # d-chunks (4)
NKB = S // P  # key blocks per batch (16)
NG = S // 512  # query groups of 512 (4)
SCALE = 1.0 / np.sqrt(np.float32(D))  # 1/22.627
MASK_VAL = -60000.0  # fits fp16; -60000/22.6 -> exp underflows to 0

F32 = mybir.dt.float32
F16 = mybir.dt.float16


def _build_attention(ctx: ExitStack, tc: tile.TileContext, out_ap, q_ap, k_ap, v_ap):
    nc = tc.nc

    consts = ctx.enter_context(tc.tile_pool(name="consts", bufs=1))
    stage = ctx.enter_context(tc.tile_pool(name="stage", bufs=8))
    kt_pool = ctx.enter_context(tc.tile_pool(name="kt", bufs=2))
    qt_pool = ctx.enter_context(tc.tile_pool(name="qt", bufs=2))
    v_pool = ctx.enter_context(tc.tile_pool(name="v", bufs=2))
    pt_pool = ctx.enter_context(tc.tile_pool(name="pt", bufs=2))
    o_pool = ctx.enter_context(tc.tile_pool(name="o", bufs=4))
    small = ctx.enter_context(tc.tile_pool(name="small", bufs=4))
    ps_st = ctx.enter_context(tc.tile_pool(name="ps_st", bufs=2, space="PSUM"))
    ps_tp = ctx.enter_context(tc.tile_pool(name="ps_tp", bufs=2, space="PSUM"))
    ps_o1 = ctx.enter_context(tc.tile_pool(name="ps_o1", bufs=2, space="PSUM"))
    ps_o2 = ctx.enter_context(tc.tile_pool(name="ps_o2", bufs=2, space="PSUM"))

    # ---- Stage all input DMAs up front, in need-order ----------------------
    # Everything shares the SWDGE ring. Order per batch: K chunks, consts
    # (batch 0 only, so ident is ready when the K transposes start), then Q
    # and V chunks interleaved in the order phases consume them. Staging
    # tiles hold 4 row-blocks each ([p, kb, d]) so one cast-DMA (fp32->fp16)
    # covers 4 blocks.
    ident = consts.tile([P, P], F16)
    umask = consts.tile([P, P], F16)
    knats = {}
    qnats = {}
    v_sbs = {}

    def _load_chunk(ap, b, c, tag):
        t_ = stage.tile([P, 4, D], F16, tag=tag)
        nc.gpsimd.dma_start(
            out=t_,
            in_=ap[b, c * P : (c + 4) * P, :].rearrange("(kb p) d -> p kb d", p=P),
        )
        return t_

    for b in range(B_LOC):
        knats[b] = [_load_chunk(k_ap, b, kc, "knat") for kc in range(0, NKB, 4)]
        if b == 0:
            # Identity for PE transposes; strictly-upper-triangular causal
            # mask U (U.T @ I adds U[qq, kk] to S^T[kk, qq], masking key
            # kk > query qq within the diagonal block).
            make_identity(nc, ident)
            make_causal_mask(nc, umask, mask_val=MASK_VAL)
        # v_sb[:, j, 0:2] = 1.0 (softmax-denominator cols), [:, j, 2:] = V.
        v_sb = v_pool.tile([P, NKB, D + 2], F16)
        v_sbs[b] = v_sb
        nc.vector.memset(v_sb[:, :, 0:2], 1.0)
        qnats[b] = []
        for c in range(0, NKB, 4):
            qnats[b].append(_load_chunk(q_ap, b, c, "qnat"))
            nc.gpsimd.dma_start(
                out=v_sb[:, c : c + 4, 2:],
                in_=v_ap[b, c * P : (c + 4) * P, :].rearrange(
                    "(kb p) d -> p kb d", p=P
                ),
            )

    def _ktp(b):
        # Build K^T [d_part, dc, keys] via PE transposes.
        kt = kt_pool.tile([P, DC, S], F16)
        for kb in range(NKB):
            tp = ps_tp.tile([P, DC, P], F16)
            for dc in range(DC):
                nc.tensor.transpose(
                    tp[:, dc, :],
                    knats[b][kb // 4][:, kb % 4, dc * P : (dc + 1) * P],
                    ident,
                )
            nc.vector.tensor_copy(kt[:, :, kb * P : (kb + 1) * P], tp)
        return kt

    def _qtp(b, g):
        # Build Q^T [d_part, dc, q_local] for query group g (512 queries).
        qt = qt_pool.tile([P, DC, 512], F16)
        for t in range(4):
            qb = 4 * g + t
            tp = ps_tp.tile([P, DC, P], F16)
            for dc in range(DC):
                nc.tensor.transpose(
                    tp[:, dc, :],
                    qnats[b][qb // 4][:, qb % 4, dc * P : (dc + 1) * P],
                    ident,
                )
            nc.vector.tensor_copy(qt[:, :, t * P : (t + 1) * P], tp)
        return qt

    kt = _ktp(0)
    qt = _qtp(0, 0)
    for b in range(B_LOC):
        v_sb = v_sbs[b]
        for g in range(NG):
            # ---- Phase A: S^T = K^T.T @ Q^T per key block; mask; exp --------
            pt = pt_pool.tile([P, NKB, 512], F16)  # [k_part, j, q_local]
            for j in range(4 * g + 4):
                o_off = max(0, (j - 4 * g) * P)  # first allowed local query
                w = 512 - o_off
                st = ps_st.tile([P, 512], F32)
                diag = j >= 4 * g
                for dc in range(DC):
                    nc.tensor.matmul(
                        st[:, :w],
                        kt[:, dc, j * P : (j + 1) * P],
                        qt[:, dc, o_off:512],
                        start=(dc == 0),
                        stop=(dc == DC - 1 and not diag),
                    )
                if diag:  # in-block causal mask via accumulating matmul
                    nc.tensor.matmul(
                        st[:, 0:P], umask, ident, start=False, stop=True
                    )
                nc.scalar.activation(
                    pt[:, j, o_off:512],
                    st[:, :w],
                    mybir.ActivationFunctionType.Exp,
                    bias=0.0,
                    scale=float(SCALE),
                )

            # Prefetch the next group's (or batch's) transposes ahead of
            # phase B.
            next_kt = next_qt = None
            if g + 1 < NG:
                next_qt = _qtp(b, g + 1)
            elif b + 1 < B_LOC:
                next_kt = _ktp(b + 1)
                next_qt = _qtp(b + 1, 0)

            # ---- Phase B: [sums|O] = P^T.T @ [1|V]; normalize; store --------
            # Split into N=258 and N=256 matmuls (one PSUM bank each); the
            # first two columns of bank 1 are the softmax denominators.
            for t in range(4):
                i = 4 * g + t  # global query tile
                o1 = ps_o1.tile([P, 258], F32)
                o2 = ps_o2.tile([P, 256], F32)
                for j in range(i + 1):
                    lhsT = pt[:, j, t * P : (t + 1) * P]
                    nc.tensor.matmul(
                        o1, lhsT, v_sb[:, j, 0:258], start=(j == 0), stop=(j == i)
                    )
                    nc.tensor.matmul(
                        o2, lhsT, v_sb[:, j, 258:514], start=(j == 0), stop=(j == i)
                    )
                recip = small.tile([P, 1], F32)
                nc.vector.reciprocal(recip, o1[:, 0:1])
                o_sb = o_pool.tile([P, D], F32)
                nc.vector.tensor_scalar_mul(o_sb[:, 0:256], o1[:, 2:258], recip)
                nc.vector.tensor_scalar_mul(o_sb[:, 256:512], o2, recip)
                nc.sync.dma_start(
                    out=out_ap[b, i * P : (i + 1) * P, :], in_=o_sb
                )

            if next_qt is not None:
                qt = next_qt
            if next_kt is not None:
                kt = next_kt


def build_nc():
    nc = bacc.Bacc(None, target_bir_lowering=False, debug=False)
    q = nc.dram_tensor("query", [B_LOC, S, D], F32, kind="ExternalInput").ap()
    k = nc.dram_tensor("key", [B_LOC, S, D], F32, kind="ExternalInput").ap()
    v = nc.dram_tensor("value", [B_LOC, S, D], F32, kind="ExternalInput").ap()
    out = nc.dram_tensor("out", [B_LOC, S, D], F32, kind="ExternalOutput").ap()
    with tile.TileContext(nc) as tc:
        with ExitStack() as ctx:
            _build_attention(ctx, tc, out, q, k, v)
    nc.compile()
    return nc


def kernel(query, key, value, _trace=False):
    query = np.ascontiguousarray(query, dtype=np.float32)
    key = np.ascontiguousarray(key, dtype=np.float32)
    value = np.ascontiguousarray(value, dtype=np.float32)
    nc = build_nc()
    in_maps = [
        {
            "query": query[c * B_LOC : (c + 1) * B_LOC],
            "key": key[c * B_LOC : (c + 1) * B_LOC],
            "value": value[c * B_LOC : (c + 1) * B_LOC],
        }
        for c in range(N_CORES)
    ]
    res = run_bass_kernel_spmd(nc, in_maps, list(range(N_CORES)), trace=_trace)
    out = np.concatenate([res.results[c]["out"] for c in range(N_CORES)], axis=0)
    if _trace:
        return out, res
    return out


# revision 23
# speedup vs baseline: 1.5798x; 1.0348x over previous
"""Causal attention kernel for Trainium2 (Bass/Tile), batch-sharded over 8 cores.

Reference computation (per batch b):
    S = Q @ K^T                  [S, S]
    S -= triu(ones, k=1) * 1e10  (causal mask, applied before scaling)
    P = softmax(S / sqrt(512), axis=-1)
    O = P @ V                    [S, D]

Shapes: B=16, S=2048, D=512, fp32. Each of the 8 cores handles 2 batches.

Design notes:
  - Matmul operands are fp16 (accumulation in PSUM stays fp32): fp32/fp32r
    matmuls pay a serialized internal weight load per instruction (~107ns),
    which dominated with many 128-wide stationary blocks. fp16 matmuls get a
    separately-pipelined LDWEIGHTS at the same 1 column/cycle stream rate.
    fp16 over bf16 for its 10-bit mantissa; all intermediate ranges (logits
    <~8 after scaling, exp <~250, inputs ~N(0,1)) fit fp16 comfortably.
  - S^T layout ([keys, queries]) so the exp output P^T feeds the PV matmul
    directly as the stationary operand; no per-tile transposes of P.
  - No max-subtraction in the softmax: logits after scaling are ~N(0,1)
    (|logit| < ~8), exp cannot overflow.
  - Softmax denominators come from two ones-columns prepended to V; the PV
    accumulation produces [sum, sum, O[:, :256]] + [O[:, 256:]] in two PSUM
    banks (fp32 bank limit is 512 columns per matmul).
  - The in-block causal mask is applied by an extra accumulating matmul
    (U.T @ I adds U[qq, kk] to S^T[kk, qq]) instead of a DVE pass over PSUM.
  - Q^T / K^T are built on-chip with PE transposes (d must sit on partitions
    for both QK^T operands); the next group's transposes are emitted before
    the current phase B so the PE fills the exp-wait bubble and the DVE
    copybacks stay ahead of the normalize ops.
  - All input DMAs are issued up front in need-order on the SWDGE ring
    (K first, then Q/V interleaved) — queueing V ahead of staging delayed
    the first PE work by ~20us in earlier revisions.
"""

import sys

sys.path.insert(0, "/opt/trn_rl_repo")

from contextlib import ExitStack

import numpy as np

import concourse.bass as bass
import concourse.tile as tile
from concourse import bacc, mybir
from concourse.bass_utils import run_bass_kernel_spmd
from concourse.masks import make_causal_mask, make_identity

N_CORES = 8
B_FULL = 16
B_LOC = B_FULL // N_CORES  # batches per core
S = 2048
D = 512
P = 128  # partitions
DC = D // P  # d-chunks (4)
NKB = S // P  # key blocks per batch (16)
NG = S // 512  # query groups of 512 (4)
SCALE = 1.0 / np.sqrt(np.float32(D))  # 1/22.627
MASK_VAL = -60000.0  # fits fp16; -60000/22.6 -> exp underflows to 0

F32 = mybir.dt.float32
F16 = mybir.dt.float16


def _build_attention(ctx: ExitStack, tc: tile.TileContext, out_ap, q_ap, k_ap, v_ap):
    nc = tc.nc

    consts = ctx.enter_context(tc.tile_pool(name="consts", bufs=1))
    stage = ctx.enter_context(tc.tile_pool(name="stage", bufs=8))
    kt_pool = ctx.enter_context(tc.tile_pool(name="kt", bufs=2))
    qt_pool = ctx.enter_context(tc.tile_pool(name="qt", bufs=2))
    v_pool = ctx.enter_context(tc.tile_pool(name="v", bufs=2))
    pt_pool = ctx.enter_context(tc.tile_pool(name="pt", bufs=2))
    o_pool = ctx.enter_context(tc.tile_pool(name="o", bufs=4))
    small = ctx.enter_context(tc.tile_pool(name="small", bufs=4))
    ps_st = ctx.enter_context(tc.tile_pool(name="ps_st", bufs=2, space="PSUM"))
    ps_tp = ctx.enter_context(tc.tile_pool(name="ps_tp", bufs=2, space="PSUM"))
    ps_o1 = ctx.enter_context(tc.tile_pool(name="ps_o1", bufs=2, space="PSUM"))
    ps_o2 = ctx.enter_context(tc.tile_pool(name="ps_o2", bufs=2, space="PSUM"))

    # ---- Stage all input DMAs up front, in need-order ----------------------
    # Everything shares the SWDGE ring. Order per batch: K chunks, consts
    # (batch 0 only, so ident is ready when the K transposes start), then Q
    # and V chunks interleaved in the order phases consume them. Staging
    # tiles hold 4 row-blocks each ([p, kb, d]) so one cast-DMA (fp32->fp16)
    # covers 4 blocks.
    ident = consts.tile([P, P], F16)
    umask = consts.tile([P, P], F16)
    knats = {}
    qnats = {}
    v_sbs = {}

    def _load_chunk(ap, b, c, tag):
        t_ = stage.tile([P, 4, D], F16, tag=tag)
        nc.gpsimd.dma_start(
            out=t_,
            in_=ap[b, c * P : (c + 4) * P, :].rearrange("(kb p) d -> p kb d", p=P),
        )
        return t_

    for b in range(B_LOC):
        knats[b] = [_load_chunk(k_ap, b, kc, "knat") for kc in range(0, NKB, 4)]
        if b == 0:
            # Identity for PE transposes; strictly-upper-triangular causal
            # mask U (U.T @ I adds U[qq, kk] to S^T[kk, qq], masking key
            # kk > query qq within the diagonal block).
            make_identity(nc, ident)
            make_causal_mask(nc, umask, mask_val=MASK_VAL)
        # v_sb[:, j, 0:2] = 1.0 (softmax-denominator cols), [:, j, 2:] = V.
        v_sb = v_pool.tile([P, NKB, D + 2], F16)
        v_sbs[b] = v_sb
        nc.vector.memset(v_sb[:, :, 0:2], 1.0)
        # Q chunk c feeds the group-c transposes (prefetched during phase
        # A(c-1)); V chunk c is first read in phase B(c). Keep each Q chunk
        # one slot ahead of the V chunk with the same index.
        def _load_v_chunk(c):
            nc.gpsimd.dma_start(
                out=v_sb[:, c : c + 4, 2:],
                in_=v_ap[b, c * P : (c + 4) * P, :].rearrange(
                    "(kb p) d -> p kb d", p=P
                ),
            )

        qnats[b] = [_load_chunk(q_ap, b, 0, "qnat")]
        for c in range(4, NKB, 4):
            qnats[b].append(_load_chunk(q_ap, b, c, "qnat"))
            _load_v_chunk(c - 4)
        _load_v_chunk(NKB - 4)

    def _ktp(b):
        # Build K^T [d_part, dc, keys] via PE transposes.
        kt = kt_pool.tile([P, DC, S], F16)
        for kb in range(NKB):
            tp = ps_tp.tile([P, DC, P], F16)
            for dc in range(DC):
                nc.tensor.transpose(
                    tp[:, dc, :],
                    knats[b][kb // 4][:, kb % 4, dc * P : (dc + 1) * P],
                    ident,
                )
            nc.vector.tensor_copy(kt[:, :, kb * P : (kb + 1) * P], tp)
        return kt

    def _qtp(b, g):
        # Build Q^T [d_part, dc, q_local] for query group g (512 queries).
        qt = qt_pool.tile([P, DC, 512], F16)
        for t in range(4):
            qb = 4 * g + t
            tp = ps_tp.tile([P, DC, P], F16)
            for dc in range(DC):
                nc.tensor.transpose(
                    tp[:, dc, :],
                    qnats[b][qb // 4][:, qb % 4, dc * P : (dc + 1) * P],
                    ident,
                )
            nc.vector.tensor_copy(qt[:, :, t * P : (t + 1) * P], tp)
        return qt

    kt = _ktp(0)
    qt = _qtp(0, 0)
    for b in range(B_LOC):
        v_sb = v_sbs[b]
        for g in range(NG):
            # ---- Phase A: S^T = K^T.T @ Q^T per key block; mask; exp --------
            pt = pt_pool.tile([P, NKB, 512], F16)  # [k_part, j, q_local]
            for j in range(4 * g + 4):
                o_off = max(0, (j - 4 * g) * P)  # first allowed local query
                w = 512 - o_off
                st = ps_st.tile([P, 512], F32)
                diag = j >= 4 * g
                for dc in range(DC):
                    nc.tensor.matmul(
                        st[:, :w],
                        kt[:, dc, j * P : (j + 1) * P],
                        qt[:, dc, o_off:512],
                        start=(dc == 0),
                        stop=(dc == DC - 1 and not diag),
                    )
                if diag:  # in-block causal mask via accumulating matmul
                    nc.tensor.matmul(
                        st[:, 0:P], umask, ident, start=False, stop=True
                    )
                nc.scalar.activation(
                    pt[:, j, o_off:512],
                    st[:, :w],
                    mybir.ActivationFunctionType.Exp,
                    bias=0.0,
                    scale=float(SCALE),
                )

            # Prefetch the next group's (or batch's) transposes ahead of
            # phase B.
            next_kt = next_qt = None
            if g + 1 < NG:
                next_qt = _qtp(b, g + 1)
            elif b + 1 < B_LOC:
                next_kt = _ktp(b + 1)
                next_qt = _qtp(b + 1, 0)

            # ---- Phase B: [sums|O] = P^T.T @ [1|V]; normalize; store --------
            # Split into N=258 and N=256 matmuls (one PSUM bank each); the
            # first two columns of bank 1 are the softmax denominators.
            for t in range(4):
                i = 4 * g + t  # global query tile
                o1 = ps_o1.tile([P, 258], F32)
                o2 = ps_o2.tile([P, 256], F32)
                for j in range(i + 1):
                    lhsT = pt[:, j, t * P : (t + 1) * P]
                    nc.tensor.matmul(
                        o1, lhsT, v_sb[:, j, 0:258], start=(j == 0), stop=(j == i)
                    )
                    nc.tensor.matmul(
                        o2, lhsT, v_sb[:, j, 258:514], start=(j == 0), stop=(j == i)
                    )
                recip = small.tile([P, 1], F32)
                nc.vector.reciprocal(recip, o1[:, 0:1])
                o_sb = o_pool.tile([P, D], F32)
                nc.vector.tensor_scalar_mul(o_sb[:, 0:256], o1[:, 2:258], recip)
                nc.vector.tensor_scalar_mul(o_sb[:, 256:512], o2, recip)
                nc.sync.dma_start(
                    out=out_ap[b, i * P : (i + 1) * P, :], in_=o_sb
                )

            if next_qt is not None:
                qt = next_qt
            if next_kt is not None:
                kt = next_kt


def build_nc():
    nc = bacc.Bacc(None, target_bir_lowering=False, debug=False)
    q = nc.dram_tensor("query", [B_LOC, S, D], F32, kind="ExternalInput").ap()
    k = nc.dram_tensor("key", [B_LOC, S, D], F32, kind="ExternalInput").ap()
    v = nc.dram_tensor("value", [B_LOC, S, D], F32, kind="ExternalInput").ap()
    out = nc.dram_tensor("out", [B_LOC, S, D], F32, kind="ExternalOutput").ap()
    with tile.TileContext(nc) as tc:
        with ExitStack() as ctx:
            _build_attention(ctx, tc, out, q, k, v)
    nc.compile()
    return nc


def kernel(query, key, value, _trace=False):
    query = np.ascontiguousarray(query, dtype=np.float32)
    key = np.ascontiguousarray(key, dtype=np.float32)
    value = np.ascontiguousarray(value, dtype=np.float32)
    nc = build_nc()
    in_maps = [
        {
            "query": query[c * B_LOC : (c + 1) * B_LOC],
            "key": key[c * B_LOC : (c + 1) * B_LOC],
            "value": value[c * B_LOC : (c + 1) * B_LOC],
        }
        for c in range(N_CORES)
    ]
    res = run_bass_kernel_spmd(nc, in_maps, list(range(N_CORES)), trace=_trace)
    out = np.concatenate([res.results[c]["out"] for c in range(N_CORES)], axis=0)
    if _trace:
        return out, res
    return out
